# revision 6
# baseline (speedup 1.0000x reference)
"""BertSelfAttention on 8 Trainium2 NeuronCores.

Problem: B=2, S=2048, H=1024, 16 heads x 64. Sharding: batch x head-group
(2 batches x 4 head-groups of 4 heads = 8 cores). Each core computes
q/k/v projections for its 4 heads and full attention over them.

The kernel is ACT(exp)-bound: 128 EXP activations of [128,1024] ~= 152us.
Structure keeps the scalar engine saturated from ~15us on:
  prefix: minimal projections (k0 chunk0, q0 cols 0-1024) so the first
    scores land ASAP.
  stage 1 (hp0,qc0): scores+exp stream; remaining k0/q0 chunks, all 16
    v-projection chains and the first k1 chunks woven into PE slack.
    PV is deferred (probs parked in a deep SBUF pb ring).
  stages 2-4: scores+exp stream for the next (hp,qc) block while the
    previous block's PV matmuls catch up 2-kc per block; tails (PSUM ->
    fp16 -> DMA-xbar transpose -> 1/denom scale -> HBM) and leftover
    q1/k1 projection chains run in the gaps. Projection chains reuse the
    PV PSUM slots (tag sharing) since PSUM is fully booked:
    2x[128,1024] score tiles + 2x[65,1024] PV accumulators = 8 banks.

Per-core device kernel (SPMD; matmul operands fp16, accumulation fp32):
  inputs (host-prepared):
    xT    [1024, 2048]  x[b].T, fp16
    wqT/wkT/wvT [1024, 256]  W.T columns for this head group, fp16
    bqk   [128, 4]      q/k biases per o-chunk (per-partition layout)
    bvb   [128, 260]    v bias + ones column, broadcast across partitions
    mb    [128, 16]     additive mask bias per key position ((1-m)*-1e30)
  output:
    out   [2048, 256]   attention output, natural [s, head-local o]

  The 65th column of each head's V block is 1.0 so the PV matmul also
  accumulates the softmax denominator (no max subtraction: scores are
  ~N(0,1), exp range is safe). Tail divides by it after a DMA-xbar
  transpose of the fp16 [65, q] accumulator to [q, 65].
"""

import sys

sys.path.insert(0, "/opt/trn_rl_repo")

import numpy as np

import concourse.bass as bass
import concourse.tile as tile
from concourse import bacc, mybir
from concourse.bass_utils import run_bass_kernel_spmd

F32 = mybir.dt.float32
F16 = mybir.dt.float16
EXP = mybir.ActivationFunctionType.Exp

B, S, H = 2, 2048, 1024
NH, HD = 16, 64
G = 4                 # head-groups (cores per batch)
NHL = NH // G         # heads per core
O = NHL * HD          # 256 output features per core
IC = H // 128         # 8 contraction chunks
KC = S // 128         # 16 key chunks
Q = 1024              # q processed in chunks of 1024
NQ = S // Q
NJ = Q // 128
NEG = -1.0e30
PBB = 36              # pb ring depth (SBUF fp16 prob tiles)


def build_nc():
    nc = bacc.Bacc(None, target_bir_lowering=False)
    xT = nc.declare_dram_parameter("xT", [H, S], F16, isOutput=False)
    wqT = nc.declare_dram_parameter("wqT", [H, O], F16, isOutput=False)
    wkT = nc.declare_dram_parameter("wkT", [H, O], F16, isOutput=False)
    wvT = nc.declare_dram_parameter("wvT", [H, O], F16, isOutput=False)
    bqk = nc.declare_dram_parameter("bqk", [128, 4], F32, isOutput=False)
    bvb = nc.declare_dram_parameter("bvb", [128, NHL * (HD + 1)], F16,
                                    isOutput=False)
    mb = nc.declare_dram_parameter("mb", [128, KC], F32, isOutput=False)
    out = nc.declare_dram_parameter("out", [S, O], F32, isOutput=True)

    with tile.TileContext(nc) as tc:
        with tc.tile_pool(name="consts", bufs=1) as consts, \
             tc.tile_pool(name="persist", bufs=1) as persist, \
             tc.tile_pool(name="xtw", bufs=1) as xtw, \
             tc.tile_pool(name="pbp", bufs=1) as pbp, \
             tc.tile_pool(name="tailp", bufs=1) as tailp, \
             tc.tile_pool(name="scps", bufs=1, space="PSUM") as scps, \
             tc.tile_pool(name="pvps", bufs=1, space="PSUM") as pvps:
            mb_sb = consts.tile([128, KC], F32, tag="mb")
            bqk_sb = consts.tile([128, 4], F32, tag="bqk")
            bvb_sb = consts.tile([128, NHL * (HD + 1)], F16, tag="bvb")

            qT = [persist.tile([128, S], F16, tag=f"qT{i}", name=f"qT{i}")
                  for i in range(2)]
            kT = [persist.tile([128, S], F16, tag=f"kT{i}", name=f"kT{i}")
                  for i in range(2)]
            vS = [persist.tile([128, NHL * (HD + 1)], F16, tag=f"v{i}",
                               name=f"v{i}") for i in range(KC)]

            xt = [xtw.tile([128, S], F16, tag=f"xt{i}", name=f"xt{i}")
                  for i in range(IC)]
            wq = [xtw.tile([128, O], F16, tag=f"wq{i}", name=f"wq{i}")
                  for i in range(IC)]
            wk = [xtw.tile([128, O], F16, tag=f"wk{i}", name=f"wk{i}")
                  for i in range(IC)]
            wv = [xtw.tile([128, O], F16, tag=f"wv{i}", name=f"wv{i}")
                  for i in range(IC)]

            # x + k/q weights first (prefix needs them), then consts +
            # ACT exp-table warmup, v weights last
            for i in range(IC):
                nc.sync.dma_start(out=xt[i], in_=xT[i * 128:(i + 1) * 128, :])
                nc.sync.dma_start(out=wk[i], in_=wkT[i * 128:(i + 1) * 128, :])
                nc.sync.dma_start(out=wq[i], in_=wqT[i * 128:(i + 1) * 128, :])
            nc.sync.dma_start(out=mb_sb, in_=mb[:, :])
            nc.sync.dma_start(out=bqk_sb, in_=bqk[:, :])
            nc.sync.dma_start(out=bvb_sb, in_=bvb[:, :])
            dummy = consts.tile([128, 1], F32, tag="dummy")
            nc.vector.memset(dummy, 0.0)
            nc.scalar.activation(dummy, dummy, EXP)
            for i in range(IC):
                nc.sync.dma_start(out=wv[i], in_=wvT[i * 128:(i + 1) * 128, :])

            # ---- projection chains (share the PV PSUM slots) ----
            slot = [0]

            def next_slot():
                slot[0] ^= 1
                return slot[0]

            def qk_chain(wt, ot, sc, dest, bcol):
                ps = pvps.tile([128, 512], F32, tag=f"pv{next_slot()}",
                               name=f"qkc{bcol}_{sc}")
                for i in range(IC):
                    nc.tensor.matmul(
                        ps,
                        lhsT=wt[i][:, ot * 128:(ot + 1) * 128],
                        rhs=xt[i][:, sc * 512:(sc + 1) * 512],
                        start=(i == 0), stop=(i == IC - 1))
                nc.vector.tensor_scalar_add(
                    dest[:, sc * 512:(sc + 1) * 512], ps,
                    bqk_sb[:, bcol:bcol + 1])

            bvview = bvb_sb.rearrange("p (h d) -> p h d", h=NHL)

            def v_chain(sc):
                ps = pvps.tile([128, O], F32, tag=f"pv{next_slot()}",
                               name=f"vch{sc}")
                for i in range(IC):
                    nc.tensor.matmul(
                        ps,
                        lhsT=xt[i][:, sc * 128:(sc + 1) * 128],
                        rhs=wv[i],
                        start=(i == 0), stop=(i == IC - 1))
                vview = vS[sc].rearrange("p (h d) -> p h d", h=NHL)
                nc.vector.tensor_add(
                    vview[:, :, 0:HD],
                    ps.rearrange("p (h d) -> p h d", h=NHL),
                    bvview[:, :, 0:HD])
                nc.vector.tensor_copy(
                    vview[:, :, HD:HD + 1], bvview[:, :, HD:HD + 1])

            # ---- attention building blocks ----
            pb_ref = {}

            def score_block(hp, qc, kc):
                sc_t = []
                for e in range(2):
                    t = scps.tile([128, Q], F32, tag=f"sc{e}",
                                  name=f"sc{e}_{hp}{qc}{kc}")
                    lo = e * 64
                    for n in range(2):
                        nc.tensor.matmul(
                            t[:, n * 512:(n + 1) * 512],
                            lhsT=kT[hp][lo:lo + 64, kc * 128:(kc + 1) * 128],
                            rhs=qT[hp][lo:lo + 64,
                                       qc * Q + n * 512:qc * Q + (n + 1) * 512],
                            start=True, stop=True)
                    sc_t.append(t)
                for e in range(2):
                    pb_t = pbp.tile([128, Q], F16, tag="pb", bufs=PBB,
                                    name=f"pb{hp}{qc}{kc}{e}")
                    nc.scalar.activation(pb_t, sc_t[e], EXP,
                                         bias=mb_sb[:, kc:kc + 1], scale=0.125)
                    pb_ref[(hp, qc, kc, e)] = pb_t

            pv_t = {}

            def pv_alloc(hp, qc):
                pv_t[(hp, qc)] = [
                    pvps.tile([HD + 1, Q], F32, tag=f"pv{e}",
                              name=f"pv{hp}{qc}{e}") for e in range(2)]

            def pv_emit(hp, qc, kc):
                for e in range(2):
                    hh = 2 * hp + e
                    t = pv_t[(hp, qc)][e]
                    pb_t = pb_ref.pop((hp, qc, kc, e))
                    for n in range(2):
                        nc.tensor.matmul(
                            t[:, n * 512:(n + 1) * 512],
                            lhsT=vS[kc][:, hh * 65:hh * 65 + 65],
                            rhs=pb_t[:, n * 512:(n + 1) * 512],
                            start=(kc == 0), stop=(kc == KC - 1))

            def tail(hp, qc):
                for e in range(2):
                    hh = 2 * hp + e
                    t = pv_t[(hp, qc)][e]
                    ovt = tailp.tile([80, Q], F16, tag=f"ovt{e}",
                                     name=f"ovt{hp}{qc}{e}")
                    nc.vector.memset(ovt[HD:80, :], 0.0)
                    nc.vector.tensor_copy(ovt[0:HD + 1, :], t)
                    tr = tailp.tile([128, NJ, 96], F16, tag=f"tr{e}",
                                    name=f"tr{hp}{qc}{e}")
                    for jb in range(NJ):
                        nc.sync.dma_start(
                            out=tr[:, jb, 0:80],
                            in_=ovt[:, jb * 128:(jb + 1) * 128],
                            transpose=True)
                    rc = tailp.tile([128, NJ], F32, tag=f"rc{e}",
                                    name=f"rc{hp}{qc}{e}")
                    nc.vector.reciprocal(rc, tr[:, :, HD])
                    osb = tailp.tile([128, NJ, HD], F32, tag=f"osb{e}",
                                     name=f"osb{hp}{qc}{e}")
                    for jb in range(NJ):
                        nc.vector.tensor_scalar_mul(
                            osb[:, jb, :], tr[:, jb, 0:HD], rc[:, jb:jb + 1])
                    dst = out[qc * Q:(qc + 1) * Q, hh * HD:(hh + 1) * HD]
                    dst = dst.rearrange("(j p) d -> p j d", p=128)
                    nc.sync.dma_start(out=dst, in_=osb)

            # ---- prefix: just enough projections for the first scores ----
            qk_chain(wk, 0, 0, kT[0], 2)
            qk_chain(wq, 0, 0, qT[0], 0)
            qk_chain(wq, 0, 1, qT[0], 0)

            # ---- stage 1: hp0/qc0 scores+exp; PV deferred; weave chains ----
            s1_extra = {
                0: [(qk_chain, (wk, 0, 1, kT[0], 2))],
                1: [(qk_chain, (wk, 0, 2, kT[0], 2))],
                2: [(qk_chain, (wk, 0, 3, kT[0], 2))],
                3: [(qk_chain, (wq, 0, 2, qT[0], 0))],
                4: [(qk_chain, (wq, 0, 3, qT[0], 0))],
                6: [(qk_chain, (wk, 1, 0, kT[1], 3))],
                8: [(qk_chain, (wk, 1, 1, kT[1], 3))],
            }
            for kc in range(KC):
                score_block(0, 0, kc)
                for fn, args in s1_extra.get(kc, []):
                    fn(*args)
                v_chain(kc)

            # ---- stage 2: hp0/qc1 stream; PV(0,0) catch-up; q1/k1 chains ----
            s2_extra = {
                9: [(qk_chain, (wq, 1, 0, qT[1], 1))],
                10: [(qk_chain, (wk, 1, 2, kT[1], 3))],
                11: [(qk_chain, (wq, 1, 1, qT[1], 1))],
                12: [(qk_chain, (wk, 1, 3, kT[1], 3))],
                13: [(qk_chain, (wq, 1, 2, qT[1], 1))],
                14: [(qk_chain, (wq, 1, 3, qT[1], 1))],
            }
            for kc in range(KC):
                score_block(0, 1, kc)
                if kc == 0:
                    pv_alloc(0, 0)
                if kc < 8:
                    pv_emit(0, 0, 2 * kc)
                    pv_emit(0, 0, 2 * kc + 1)
                elif kc == 8:
                    tail(0, 0)
                for fn, args in s2_extra.get(kc, []):
                    fn(*args)

            # ---- stage 3: hp1/qc0 stream; PV(0,1) then PV(1,0) catch-up ----
            for kc in range(KC):
                score_block(1, 0, kc)
                if kc == 0:
                    pv_alloc(0, 1)
                if kc < 8:
                    pv_emit(0, 1, 2 * kc)
                    pv_emit(0, 1, 2 * kc + 1)
                elif kc == 8:
                    tail(0, 1)
                else:
                    if kc == 9:
                        pv_alloc(1, 0)
                    pv_emit(1, 0, 2 * (kc - 9))
                    pv_emit(1, 0, 2 * (kc - 9) + 1)

            # ---- stage 4: hp1/qc1 stream; PV(1,0) finish; PV(1,1) lag-1 ----
            for kc in range(KC):
                if kc == 0:
                    pv_emit(1, 0, 14)
                    pv_emit(1, 0, 15)
                score_block(1, 1, kc)
                if kc == 1:
                    tail(1, 0)
                elif kc == 2:
                    pv_alloc(1, 1)
                    pv_emit(1, 1, 0)
                    pv_emit(1, 1, 1)
                elif kc == 3:
                    pv_emit(1, 1, 2)
                elif kc >= 4:
                    pv_emit(1, 1, kc - 1)
            pv_emit(1, 1, 15)
            tail(1, 1)
    nc.finalize()
    return nc


_NC_CACHE = None


def _get_nc():
    global _NC_CACHE
    if _NC_CACHE is None:
        _NC_CACHE = build_nc()
    return _NC_CACHE


def make_in_maps(inputs, attention_mask, Wq, bq, Wk, bk, Wv, bv):
    x = np.asarray(inputs, dtype=np.float32)
    mask = np.asarray(attention_mask)
    Wq = np.asarray(Wq, dtype=np.float32)
    Wk = np.asarray(Wk, dtype=np.float32)
    Wv = np.asarray(Wv, dtype=np.float32)
    bq = np.asarray(bq, dtype=np.float32)
    bk = np.asarray(bk, dtype=np.float32)
    bv = np.asarray(bv, dtype=np.float32)

    xTb = [np.ascontiguousarray(x[b].T).astype(np.float16) for b in range(B)]
    mbb = [np.ascontiguousarray(
        ((1.0 - mask[b].astype(np.float32)) * NEG).reshape(KC, 128).T)
        for b in range(B)]
    in_maps = []
    for c in range(8):
        b, g = c // G, c % G
        cols = slice(g * O, (g + 1) * O)
        bqs, bks = bq[cols], bk[cols]
        bvc = np.concatenate(
            [np.concatenate([bv[cols][h * 64:(h + 1) * 64], [1.0]])
             for h in range(NHL)]).astype(np.float32)
        bvbc = np.ascontiguousarray(np.broadcast_to(bvc[None, :], (128, len(bvc))))
        in_maps.append({
            "xT": xTb[b],
            "wqT": np.ascontiguousarray(Wq.T[:, cols]).astype(np.float16),
            "wkT": np.ascontiguousarray(Wk.T[:, cols]).astype(np.float16),
            "wvT": np.ascontiguousarray(Wv.T[:, cols]).astype(np.float16),
            "bqk": np.ascontiguousarray(
                np.stack([bqs[:128], bqs[128:], bks[:128], bks[128:]], axis=1)),
            "bvb": bvbc.astype(np.float16),
            "mb": mbb[b],
        })
    return in_maps


def assemble(results):
    outs = [results[c]["out"] for c in range(8)]
    full = np.stack(
        [np.concatenate(outs[b * G:(b + 1) * G], axis=1) for b in range(B)])
    return np.ascontiguousarray(full.astype(np.float32))


def kernel(**inputs) -> np.ndarray:
    nc = _get_nc()
    in_maps = make_in_maps(**inputs)
    res = run_bass_kernel_spmd(nc, in_maps, core_ids=list(range(8)))
    return assemble(res.results)


# revision 12
# speedup vs baseline: 1.2766x; 1.2766x over previous
"""BertSelfAttention on 8 Trainium2 NeuronCores.

Problem: B=2, S=2048, H=1024, 16 heads x 64. Sharding: batch x head-group
(2 batches x 4 head-groups of 4 heads = 8 cores). Each core computes
q/k/v projections for its 4 heads and full attention over them.

The kernel is ACT(exp)-bound: 128 EXP activations of [128,1024] ~= 152us.
Structure keeps the scalar engine saturated from ~15us on:
  prefix: minimal projections (k0 chunk0, q0 cols 0-1024) so the first
    scores land ASAP.
  stage 1 (hp0,qc0): scores+exp stream; remaining k0/q0 chunks, all 16
    v-projection chains and the first k1 chunks woven into PE slack.
    PV is deferred (probs parked in a deep SBUF pb ring).
  stages 2-4: scores+exp stream for the next (hp,qc) block while the
    previous block's PV matmuls catch up 2-kc per block; tails (PSUM ->
    SBUF -> PE transpose -> 1/denom scale -> HBM) and leftover q1/k1
    projection chains run in the gaps. Projection chains and the tail
    transposes reuse the PV PSUM slots (tag sharing) since PSUM is fully
    booked: 2x[128,1024] score tiles + 2x[65,1024] PV accumulators = 8
    banks.

Per-core device kernel (SPMD; matmul operands fp16, accumulation fp32):
  inputs (host-prepared):
    xT    [1024, 2048]  x[b].T, fp16
    wqT/wkT/wvT [1024, 256]  W.T columns for this head group, fp16
    bqk   [128, 4]      q/k biases per o-chunk (per-partition layout)
    bvb   [128, 260]    v bias + ones column, broadcast across partitions
    mb    [128, 16]     additive mask bias per key position ((1-m)*-1e30)
  output:
    out   [2048, 256]   attention output, natural [s, head-local o]

  The 65th column of each head's V block is 1.0 so the PV matmul also
  accumulates the softmax denominator (no max subtraction: scores are
  ~N(0,1), exp range is safe). Tail divides by it after a DMA-xbar
  transpose of the fp16 [65, q] accumulator to [q, 65].
"""

import sys

sys.path.insert(0, "/opt/trn_rl_repo")

import numpy as np

import concourse.bass as bass
import concourse.tile as tile
from concourse.masks import make_identity
from concourse import bacc, mybir
from concourse.bass_utils import run_bass_kernel_spmd

F32 = mybir.dt.float32
F16 = mybir.dt.float16
EXP = mybir.ActivationFunctionType.Exp

B, S, H = 2, 2048, 1024
NH, HD = 16, 64
G = 4                 # head-groups (cores per batch)
NHL = NH // G         # heads per core
O = NHL * HD          # 256 output features per core
IC = H // 128         # 8 contraction chunks
KC = S // 128         # 16 key chunks
Q = 1024              # q processed in chunks of 1024
NQ = S // Q
NJ = Q // 128
NEG = -1.0e30
PBB = 36              # pb ring depth (SBUF fp16 prob tiles)


def build_nc():
    nc = bacc.Bacc(None, target_bir_lowering=False)
    xT = nc.declare_dram_parameter("xT", [H, S], F16, isOutput=False)
    wqT = nc.declare_dram_parameter("wqT", [H, O], F16, isOutput=False)
    wkT = nc.declare_dram_parameter("wkT", [H, O], F16, isOutput=False)
    wvT = nc.declare_dram_parameter("wvT", [H, O], F16, isOutput=False)
    bqk = nc.declare_dram_parameter("bqk", [128, 4], F32, isOutput=False)
    bvb = nc.declare_dram_parameter("bvb", [128, NHL * (HD + 1)], F16,
                                    isOutput=False)
    mb = nc.declare_dram_parameter("mb", [128, KC], F32, isOutput=False)
    out = nc.declare_dram_parameter("out", [S, O], F32, isOutput=True)

    with tile.TileContext(nc) as tc:
        with tc.tile_pool(name="consts", bufs=1) as consts, \
             tc.tile_pool(name="persist", bufs=1) as persist, \
             tc.tile_pool(name="xtw", bufs=1) as xtw, \
             tc.tile_pool(name="pbp", bufs=1) as pbp, \
             tc.tile_pool(name="tailp", bufs=1) as tailp, \
             tc.tile_pool(name="scps", bufs=1, space="PSUM") as scps, \
             tc.tile_pool(name="pvps", bufs=1, space="PSUM") as pvps:
            ident = consts.tile([128, 128], F32, tag="ident")
            make_identity(nc, ident)
            mb_sb = consts.tile([128, KC], F32, tag="mb")
            bqk_sb = consts.tile([128, 4], F32, tag="bqk")
            bvb_sb = consts.tile([128, NHL * (HD + 1)], F16, tag="bvb")

            qT = [persist.tile([128, S], F16, tag=f"qT{i}", name=f"qT{i}")
                  for i in range(2)]
            kT = [persist.tile([128, S], F16, tag=f"kT{i}", name=f"kT{i}")
                  for i in range(2)]
            vS = [persist.tile([128, NHL * (HD + 1)], F16, tag=f"v{i}",
                               name=f"v{i}") for i in range(KC)]

            xt = [xtw.tile([128, S], F16, tag=f"xt{i}", name=f"xt{i}")
                  for i in range(IC)]
            wq = [xtw.tile([128, O], F16, tag=f"wq{i}", name=f"wq{i}")
                  for i in range(IC)]
            wk = [xtw.tile([128, O], F16, tag=f"wk{i}", name=f"wk{i}")
                  for i in range(IC)]
            wv = [xtw.tile([128, O], F16, tag=f"wv{i}", name=f"wv{i}")
                  for i in range(IC)]

            # quarter the x stream so the first chains start ASAP:
            # [xt cols 0-512 + wk] -> k0sc0; [wq + xt cols 512-1024] ->
            # q0sc0/sc1; consts + ACT exp-table warmup; rest of x; wv
            for i in range(IC):
                nc.sync.dma_start(out=xt[i][:, 0:512],
                                  in_=xT[i * 128:(i + 1) * 128, 0:512])
                nc.sync.dma_start(out=wk[i], in_=wkT[i * 128:(i + 1) * 128, :])
            for i in range(IC):
                nc.sync.dma_start(out=wq[i], in_=wqT[i * 128:(i + 1) * 128, :])
                nc.sync.dma_start(out=xt[i][:, 512:1024],
                                  in_=xT[i * 128:(i + 1) * 128, 512:1024])
            nc.sync.dma_start(out=mb_sb, in_=mb[:, :])
            nc.sync.dma_start(out=bqk_sb, in_=bqk[:, :])
            nc.sync.dma_start(out=bvb_sb, in_=bvb[:, :])
            dummy = consts.tile([128, 1], F32, tag="dummy")
            nc.vector.memset(dummy, 0.0)
            nc.scalar.activation(dummy, dummy, EXP)
            for i in range(IC):
                nc.sync.dma_start(out=xt[i][:, 1024:1536],
                                  in_=xT[i * 128:(i + 1) * 128, 1024:1536])
                nc.sync.dma_start(out=wv[i], in_=wvT[i * 128:(i + 1) * 128, :])
            for i in range(IC):
                nc.sync.dma_start(out=xt[i][:, 1536:2048],
                                  in_=xT[i * 128:(i + 1) * 128, 1536:2048])

            # ---- projection chains (share the PV PSUM slots) ----
            slot = [0]

            def next_slot():
                slot[0] ^= 1
                return slot[0]

            def qk_chain(wt, ot, sc, dest, bcol):
                ps = pvps.tile([128, 512], F32, tag=f"pv{next_slot()}",
                               name=f"qkc{bcol}_{sc}")
                for i in range(IC):
                    nc.tensor.matmul(
                        ps,
                        lhsT=wt[i][:, ot * 128:(ot + 1) * 128],
                        rhs=xt[i][:, sc * 512:(sc + 1) * 512],
                        start=(i == 0), stop=(i == IC - 1))
                nc.vector.tensor_scalar_add(
                    dest[:, sc * 512:(sc + 1) * 512], ps,
                    bqk_sb[:, bcol:bcol + 1])

            bvview = bvb_sb.rearrange("p (h d) -> p h d", h=NHL)

            def v_chain(sc):
                ps = pvps.tile([128, O], F32, tag=f"pv{next_slot()}",
                               name=f"vch{sc}")
                for i in range(IC):
                    nc.tensor.matmul(
                        ps,
                        lhsT=xt[i][:, sc * 128:(sc + 1) * 128],
                        rhs=wv[i],
                        start=(i == 0), stop=(i == IC - 1))
                vview = vS[sc].rearrange("p (h d) -> p h d", h=NHL)
                nc.vector.tensor_add(
                    vview[:, :, 0:HD],
                    ps.rearrange("p (h d) -> p h d", h=NHL),
                    bvview[:, :, 0:HD])
                nc.vector.tensor_copy(
                    vview[:, :, HD:HD + 1], bvview[:, :, HD:HD + 1])

            # ---- attention building blocks ----
            pb_ref = {}

            def score_block(hp, qc, kc):
                sc_t = []
                for e in range(2):
                    t = scps.tile([128, Q], F32, tag=f"sc{e}",
                                  name=f"sc{e}_{hp}{qc}{kc}")
                    lo = e * 64
                    for n in range(2):
                        nc.tensor.matmul(
                            t[:, n * 512:(n + 1) * 512],
                            lhsT=kT[hp][lo:lo + 64, kc * 128:(kc + 1) * 128],
                            rhs=qT[hp][lo:lo + 64,
                                       qc * Q + n * 512:qc * Q + (n + 1) * 512],
                            start=True, stop=True)
                    sc_t.append(t)
                for e in range(2):
                    pb_t = pbp.tile([128, Q], F16, tag="pb", bufs=PBB,
                                    name=f"pb{hp}{qc}{kc}{e}")
                    nc.scalar.activation(pb_t, sc_t[e], EXP,
                                         bias=mb_sb[:, kc:kc + 1], scale=0.125)
                    pb_ref[(hp, qc, kc, e)] = pb_t

            pv_t = {}

            def pv_alloc(hp, qc):
                pv_t[(hp, qc)] = [
                    pvps.tile([HD + 1, Q], F32, tag=f"pv{e}",
                              name=f"pv{hp}{qc}{e}") for e in range(2)]

            def pv_emit(hp, qc, kc):
                for e in range(2):
                    hh = 2 * hp + e
                    t = pv_t[(hp, qc)][e]
                    pb_t = pb_ref.pop((hp, qc, kc, e))
                    for n in range(2):
                        nc.tensor.matmul(
                            t[:, n * 512:(n + 1) * 512],
                            lhsT=vS[kc][:, hh * 65:hh * 65 + 65],
                            rhs=pb_t[:, n * 512:(n + 1) * 512],
                            start=(kc == 0), stop=(kc == KC - 1))

            def tail(hp, qc):
                for e in range(2):
                    hh = 2 * hp + e
                    t = pv_t[(hp, qc)][e]
                    ovt = tailp.tile([HD + 1, Q], F32, tag=f"ovt{e}",
                                     name=f"ovt{hp}{qc}{e}")
                    nc.vector.tensor_copy(ovt, t)
                    # PE transpose into the just-freed pv slot (tag reuse)
                    tr = pvps.tile([128, NJ, 128], F32, tag=f"pv{e}",
                                   name=f"tr{hp}{qc}{e}")
                    for jb in range(NJ):
                        nc.tensor.transpose(
                            tr[:, jb, 0:HD + 1],
                            ovt[:, jb * 128:(jb + 1) * 128],
                            ident[0:HD + 1, 0:HD + 1])
                    rc = tailp.tile([128, NJ], F32, tag=f"rc{e}",
                                    name=f"rc{hp}{qc}{e}")
                    nc.vector.reciprocal(rc, tr[:, :, HD])
                    osb = tailp.tile([128, NJ, HD], F32, tag=f"osb{e}",
                                     name=f"osb{hp}{qc}{e}")
                    for jb in range(NJ):
                        nc.vector.tensor_scalar_mul(
                            osb[:, jb, :], tr[:, jb, 0:HD], rc[:, jb:jb + 1])
                    dst = out[qc * Q:(qc + 1) * Q, hh * HD:(hh + 1) * HD]
                    dst = dst.rearrange("(j p) d -> p j d", p=128)
                    nc.sync.dma_start(out=dst, in_=osb)

            # ---- prefix: just enough projections for the first scores ----
            qk_chain(wk, 0, 0, kT[0], 2)
            qk_chain(wq, 0, 0, qT[0], 0)
            qk_chain(wq, 0, 1, qT[0], 0)

            # ---- stage 1: hp0/qc0 scores+exp; PV deferred; weave chains ----
            s1_extra = {
                0: [(qk_chain, (wk, 0, 1, kT[0], 2))],
                2: [(qk_chain, (wk, 0, 2, kT[0], 2))],
                4: [(qk_chain, (wk, 0, 3, kT[0], 2))],
                6: [(qk_chain, (wq, 0, 2, qT[0], 0))],
                8: [(qk_chain, (wq, 0, 3, qT[0], 0))],
                10: [(qk_chain, (wk, 1, 0, kT[1], 3))],
                12: [(qk_chain, (wk, 1, 1, kT[1], 3))],
            }
            # v chains spread over blocks 1..15 (doubling up where needed)
            s1_v = {1: [0], 2: [1], 3: [2], 4: [3], 5: [4], 6: [5], 7: [6],
                    8: [7], 9: [8], 10: [9], 11: [10, 11], 12: [12],
                    13: [13], 14: [14], 15: [15]}
            for kc in range(KC):
                score_block(0, 0, kc)
                for fn, args in s1_extra.get(kc, []):
                    fn(*args)
                for v in s1_v.get(kc, []):
                    v_chain(v)

            # ---- stage 2: hp0/qc1 stream; PV(0,0) catch-up; q1/k1 chains ----
            s2_extra = {
                9: [(qk_chain, (wq, 1, 0, qT[1], 1))],
                10: [(qk_chain, (wk, 1, 2, kT[1], 3))],
                11: [(qk_chain, (wq, 1, 1, qT[1], 1))],
                12: [(qk_chain, (wk, 1, 3, kT[1], 3))],
                13: [(qk_chain, (wq, 1, 2, qT[1], 1))],
                14: [(qk_chain, (wq, 1, 3, qT[1], 1))],
            }
            for kc in range(KC):
                score_block(0, 1, kc)
                if kc == 0:
                    pv_alloc(0, 0)
                if kc < 8:
                    pv_emit(0, 0, 2 * kc)
                    pv_emit(0, 0, 2 * kc + 1)
                elif kc == 8:
                    tail(0, 0)
                for fn, args in s2_extra.get(kc, []):
                    fn(*args)

            # ---- stage 3: hp1/qc0 stream; PV(0,1) then PV(1,0) catch-up ----
            for kc in range(KC):
                score_block(1, 0, kc)
                if kc == 0:
                    pv_alloc(0, 1)
                if kc < 8:
                    pv_emit(0, 1, 2 * kc)
                    pv_emit(0, 1, 2 * kc + 1)
                elif kc == 8:
                    tail(0, 1)
                else:
                    if kc == 9:
                        pv_alloc(1, 0)
                    pv_emit(1, 0, 2 * (kc - 9))
                    pv_emit(1, 0, 2 * (kc - 9) + 1)

            # ---- stage 4: hp1/qc1 stream; PV(1,0) finish; PV(1,1) lag-1 ----
            for kc in range(KC):
                if kc == 0:
                    pv_emit(1, 0, 14)
                    pv_emit(1, 0, 15)
                score_block(1, 1, kc)
                if kc == 1:
                    tail(1, 0)
                elif kc == 2:
                    pv_alloc(1, 1)
                    pv_emit(1, 1, 0)
                    pv_emit(1, 1, 1)
                elif kc == 3:
                    pv_emit(1, 1, 2)
                elif kc >= 4:
                    pv_emit(1, 1, kc - 1)
            pv_emit(1, 1, 15)
            tail(1, 1)
    nc.finalize()
    return nc


_NC_CACHE = None


def _get_nc():
    global _NC_CACHE
    if _NC_CACHE is None:
        _NC_CACHE = build_nc()
    return _NC_CACHE


def make_in_maps(inputs, attention_mask, Wq, bq, Wk, bk, Wv, bv):
    x = np.asarray(inputs, dtype=np.float32)
    mask = np.asarray(attention_mask)
    Wq = np.asarray(Wq, dtype=np.float32)
    Wk = np.asarray(Wk, dtype=np.float32)
    Wv = np.asarray(Wv, dtype=np.float32)
    bq = np.asarray(bq, dtype=np.float32)
    bk = np.asarray(bk, dtype=np.float32)
    bv = np.asarray(bv, dtype=np.float32)

    xTb = [np.ascontiguousarray(x[b].T).astype(np.float16) for b in range(B)]
    mbb = [np.ascontiguousarray(
        ((1.0 - mask[b].astype(np.float32)) * NEG).reshape(KC, 128).T)
        for b in range(B)]
    in_maps = []
    for c in range(8):
        b, g = c // G, c % G
        cols = slice(g * O, (g + 1) * O)
        bqs, bks = bq[cols], bk[cols]
        bvc = np.concatenate(
            [np.concatenate([bv[cols][h * 64:(h + 1) * 64], [1.0]])
             for h in range(NHL)]).astype(np.float32)
        bvbc = np.ascontiguousarray(np.broadcast_to(bvc[None, :], (128, len(bvc))))
        in_maps.append({
            "xT": xTb[b],
            "wqT": np.ascontiguousarray(Wq.T[:, cols]).astype(np.float16),
            "wkT": np.ascontiguousarray(Wk.T[:, cols]).astype(np.float16),
            "wvT": np.ascontiguousarray(Wv.T[:, cols]).astype(np.float16),
            "bqk": np.ascontiguousarray(
                np.stack([bqs[:128], bqs[128:], bks[:128], bks[128:]], axis=1)),
            "bvb": bvbc.astype(np.float16),
            "mb": mbb[b],
        })
    return in_maps


def assemble(results):
    outs = [results[c]["out"] for c in range(8)]
    full = np.stack(
        [np.concatenate(outs[b * G:(b + 1) * G], axis=1) for b in range(B)])
    return np.ascontiguousarray(full.astype(np.float32))


def kernel(**inputs) -> np.ndarray:
    nc = _get_nc()
    in_maps = make_in_maps(**inputs)
    res = run_bass_kernel_spmd(nc, in_maps, core_ids=list(range(8)))
    return assemble(res.results)
